# revision 1
# baseline (speedup 1.0000x reference)
"""Trainium2 Bass kernel for nn_AttentionDecoder (bf16 rewrite).

Key insight (from the reference): the per-step attention score adds a
per-batch scalar (sa) to every element of the row before softmax;
softmax is shift-invariant, so the attention weights -- and the context
vector ci -- are identical for all 64 decode steps.  The computation
collapses to:

  Phase A (streams h once):
     twh[b,t] = sum_h tanh(h[b,t,:] @ W_h_a.T)[h] * wa1[h]
     e        = exp(twh)            (unnormalized; |twh| <= ~10)
     ci[b,:]  = (e @ h[b]) / sum(e)
     s0       = tanh(h[:,0,:] @ W_init.T + b_init)
  Phase B (64 sequential GRU+RNN steps, batch=8 per core):
     si = GRU(cat(ci,y), s);  yi = softmax(tanh(RNN(cat(ci,si), y)))

Perf notes vs the fp32 baseline (1.14 ms):
  * All matmuls in bf16: fp32 matmuls run twice (fp32_mode=LOW/HIGH, two
    LDWEIGHTS+MATMUL pairs each) and disable FWL fast weight load.  bf16
    is single-pass with FWL -- production-measured ~81 ns/MM at N=128.
  * One activation table for the whole kernel (set 0: exp+tanh).  The
    wide-domain phase A softmax computes exp via the tanh identity
    exp(x) = (1+tanh(x/2))/(1-tanh(x/2)) -- the exp table is far less
    accurate than tanh's over +-10.  Phase B's softmax input is
    tanh(..) in [-1,1], where native Exp is accuracy-neutral (measured
    bit-identical) and replaces four serial vector ops.  GRU sigmoids
    are emulated as 0.5*tanh(0.5x)+0.5 to stay on one table.
  * Zero GpSimd use (baseline lost ~190us to gpsimd semaphore overhead).
    Partition reductions/broadcasts use k=1 matmuls against ones.
  * Phase B software-pipelined: step t's s-dependent gate matmuls are
    emitted before step t-1's softmax tail so the PE stays busy.  The
    RNN's ci-dependent constant is folded into its matmul group (2 extra
    matmuls) and its bias rides the tanh activation's per-partition bias
    operand, removing a vector-engine add from the serial chain; the
    softmax 1/S reciprocal writes bf16 directly (it is rounded to bf16
    for the broadcast matmul anyway), removing the cast.
  * PSUM accumulation groups are never interleaved with other groups in
    the same bank: start_tensor_calc=True wipes the bank's has-written
    state, silently dropping earlier partial sums (measured, not in the
    docs).  y contributions get their own single-matmul regions and are
    merged on the vector engine instead.

Sharding: data-parallel over batch, 8 batches per core, weights
replicated; h is cast to bf16 host-side (tolerance is 2e-2).
"""

import numpy as np
import ml_dtypes

B, T, D, H, DO, L = 64, 2048, 256, 256, 128, 64
NC = 8           # cores
BL = B // NC     # batches per core = 8
NT = T // 128    # 16 t-chunks

_CACHE = {}


def _build_program():
    import concourse.bass as bass
    import concourse.bacc as bacc
    import concourse.mybir as mybir
    import concourse.tile as tile

    dt = mybir.dt
    F32 = dt.float32
    BF16 = dt.bfloat16
    AF = mybir.ActivationFunctionType
    OP = mybir.AluOpType
    AX = mybir.AxisListType

    nc = bacc.Bacc("TRN2", target_bir_lowering=False, debug=False, num_devices=NC)

    # ---- DRAM I/O ------------------------------------------------------
    h_d = nc.dram_tensor("h", (BL, T, D), BF16, kind="ExternalInput").ap()
    whaT_d = nc.dram_tensor("whaT", (128, 512), BF16, kind="ExternalInput").ap()
    wa1r_d = nc.dram_tensor("wa1r", (128, 256), BF16, kind="ExternalInput").ap()
    winitT_d = nc.dram_tensor("winitT", (128, 512), BF16, kind="ExternalInput").ap()
    binit_d = nc.dram_tensor("binit", (128, 2), F32, kind="ExternalInput").ap()
    wgsT_d = nc.dram_tensor("wgsT", (128, 1536), BF16, kind="ExternalInput").ap()
    wgyT_d = nc.dram_tensor("wgyT", (128, 768), BF16, kind="ExternalInput").ap()
    wgciT_d = nc.dram_tensor("wgciT", (128, 1536), BF16, kind="ExternalInput").ap()
    wrsT_d = nc.dram_tensor("wrsT", (128, 256), BF16, kind="ExternalInput").ap()
    wryT_d = nc.dram_tensor("wryT", (128, 128), BF16, kind="ExternalInput").ap()
    wrciT_d = nc.dram_tensor("wrciT", (128, 256), BF16, kind="ExternalInput").ap()
    biasgT_d = nc.dram_tensor("biasgT", (128, 6), F32, kind="ExternalInput").ap()
    biasrT_d = nc.dram_tensor("biasrT", (128, 1), F32, kind="ExternalInput").ap()
    ident_d = nc.dram_tensor("ident", (128, 128), BF16, kind="ExternalInput").ap()
    onescol_d = nc.dram_tensor("onescol", (128, 1), BF16, kind="ExternalInput").ap()
    onesrow_d = nc.dram_tensor("onesrow", (1, 128), BF16, kind="ExternalInput").ap()
    out_d = nc.dram_tensor("out", (128, BL * L), BF16, kind="ExternalOutput").ap()

    # ---- persistent SBUF ----------------------------------------------
    whaT = nc.alloc_sbuf_tensor("whaT_sb", [128, 512], BF16).ap()
    wa1r = nc.alloc_sbuf_tensor("wa1r_sb", [128, 256], BF16).ap()
    winitT = nc.alloc_sbuf_tensor("winitT_sb", [128, 512], BF16).ap()
    binit = nc.alloc_sbuf_tensor("binit_sb", [128, 2], F32).ap()
    wgsT = nc.alloc_sbuf_tensor("wgsT_sb", [128, 1536], BF16).ap()
    wgyT = nc.alloc_sbuf_tensor("wgyT_sb", [128, 768], BF16).ap()
    wgciT = nc.alloc_sbuf_tensor("wgciT_sb", [128, 1536], BF16).ap()
    wrsT = nc.alloc_sbuf_tensor("wrsT_sb", [128, 256], BF16).ap()
    wryT = nc.alloc_sbuf_tensor("wryT_sb", [128, 128], BF16).ap()
    wrciT = nc.alloc_sbuf_tensor("wrciT_sb", [128, 256], BF16).ap()
    biasgT = nc.alloc_sbuf_tensor("biasgT_sb", [128, 6], F32).ap()
    biasrT = nc.alloc_sbuf_tensor("biasrT_sb", [128, 1], F32).ap()
    ident = nc.alloc_sbuf_tensor("ident_sb", [128, 128], BF16).ap()
    onescol = nc.alloc_sbuf_tensor("onescol_sb", [128, 1], BF16).ap()
    onesrow = nc.alloc_sbuf_tensor("onesrow_sb", [1, 128], BF16).ap()

    h0T = nc.alloc_sbuf_tensor("h0T", [128, 16], BF16).ap()      # h[:,0,:] cols c*8+b
    partials = nc.alloc_sbuf_tensor("partials", [128, 8], F32).ap()
    ciT = nc.alloc_sbuf_tensor("ciT", [128, 16], BF16).ap()      # cols c*8+b
    s0T = nc.alloc_sbuf_tensor("s0T", [128, 16], BF16).ap()
    constgT = nc.alloc_sbuf_tensor("constgT", [128, 48], F32).ap()
    constrT = nc.alloc_sbuf_tensor("constrT", [128, 8], F32).ap()
    out_all = nc.alloc_sbuf_tensor("out_all", [128, BL * L], BF16).ap()

    with tile.TileContext(nc) as tc:
        # weight loads
        for sb, dr in [(whaT, whaT_d), (wa1r, wa1r_d), (winitT, winitT_d),
                       (binit, binit_d), (wgsT, wgsT_d), (wgyT, wgyT_d),
                       (wgciT, wgciT_d), (wrsT, wrsT_d), (wryT, wryT_d),
                       (wrciT, wrciT_d), (biasgT, biasgT_d), (biasrT, biasrT_d),
                       (ident, ident_d), (onescol, onescol_d),
                       (onesrow, onesrow_d)]:
            nc.sync.dma_start(sb[:, :], dr[:, :])

        # ================= Phase A =================
        with tc.tile_pool(name="pcit", bufs=1, space="PSUM") as pcit_pool:
          # both ci accumulators share one bank (32B/partition each),
          # freeing a bank for a deeper transpose ring below
          pciTx = pcit_pool.tile([128, 16], F32, name="pciTx", tag="pciT")
          pciT0 = pciTx[:, 0:8]
          pciT1 = pciTx[:, 8:16]
          with tc.tile_pool(name="hnat", bufs=24) as hnat_pool, \
             tc.tile_pool(name="hts", bufs=8) as ht_pool, \
             tc.tile_pool(name="sba", bufs=6) as sba_pool, \
             tc.tile_pool(name="smalla", bufs=4) as sm_pool, \
             tc.tile_pool(name="ptr", bufs=3, space="PSUM") as ptr_pool, \
             tc.tile_pool(name="pwh", bufs=2, space="PSUM") as pwh_pool, \
             tc.tile_pool(name="pci", bufs=2, space="PSUM") as pci_pool:

            for b in range(BL):
                hn_tiles = []
                twh = sm_pool.tile([128, 16], F32, name=f"twh{b}", tag="twh")
                for i in range(NT):
                    hn = hnat_pool.tile([128, 256], BF16, name=f"hn{b}_{i}", tag="hn")
                    hn_tiles.append(hn)
                    nc.sync.dma_start(hn[:, :], h_d[b, bass.ts(i, 128), :])
                    # transpose both d-halves: (128t,128d) -> (128d,128t)
                    pt0 = ptr_pool.tile([128, 128], BF16, name=f"pt0_{b}_{i}", tag="pt")
                    pt1 = ptr_pool.tile([128, 128], BF16, name=f"pt1_{b}_{i}", tag="pt")
                    nc.tensor.transpose(pt0[:, :], hn[:, 0:128], ident[:, :])
                    nc.tensor.transpose(pt1[:, :], hn[:, 128:256], ident[:, :])
                    ht0 = ht_pool.tile([128, 128], BF16, name=f"ht0_{b}_{i}", tag="ht0")
                    ht1 = ht_pool.tile([128, 128], BF16, name=f"ht1_{b}_{i}", tag="ht1")
                    nc.vector.tensor_copy(ht0[:, :], pt0[:, :])
                    nc.scalar.copy(ht1[:, :], pt1[:, :])
                    if i == 0:
                        nc.vector.tensor_copy(h0T[:, b:b + 1], ht0[:, 0:1])
                        nc.vector.tensor_copy(h0T[:, 8 + b:8 + b + 1], ht1[:, 0:1])
                    # wh = h @ W_h_a.T for this chunk: (128t, 256h)
                    pw = pwh_pool.tile([128, 256], F32, name=f"pw{b}_{i}", tag="pw")
                    nc.tensor.matmul(pw[:, :], ht0[:, :], whaT[:, 0:256],
                                     start=True, stop=False)
                    nc.tensor.matmul(pw[:, :], ht1[:, :], whaT[:, 256:512],
                                     start=False, stop=True)
                    th = sba_pool.tile([128, 256], BF16, name=f"th{b}_{i}", tag="th")
                    nc.scalar.activation(th[:, :], pw[:, :], AF.Tanh)
                    tw = sba_pool.tile([128, 256], BF16, name=f"tw{b}_{i}", tag="tw")
                    nc.vector.tensor_mul(tw[:, :], th[:, :], wa1r[:, :])
                    nc.vector.reduce_sum(twh[:, i:i + 1], tw[:, :], axis=AX.X)

                # e = exp(twh) (unnormalized) via exp(x) = (1+t)/(1-t),
                # t = tanh(x/2): the tanh table is ~100x more accurate than
                # the exp table (act_info err 4 vs 400; native Exp measured
                # 3.9e-2 end-to-end rel err vs 2e-3 with the identity).
                tt = sm_pool.tile([128, 16], F32, name=f"tt{b}", tag="tt")
                nc.scalar.activation(tt[:, :], twh[:, :], AF.Tanh, scale=0.5)
                uu = sm_pool.tile([128, 16], F32, name=f"uu{b}", tag="uu")
                nc.vector.tensor_scalar_add(uu[:, :], tt[:, :], 1.0)
                ww = sm_pool.tile([128, 16], F32, name=f"ww{b}", tag="ww")
                nc.vector.tensor_scalar(ww[:, :], tt[:, :], -1.0, 1.0,
                                        OP.mult, OP.add)
                rw = sm_pool.tile([128, 16], F32, name=f"rw{b}", tag="rw")
                nc.vector.reciprocal(rw[:, :], ww[:, :])
                ee = sm_pool.tile([128, 16], BF16, name=f"ee{b}", tag="ee")
                nc.vector.tensor_mul(ee[:, :], uu[:, :], rw[:, :])
                nc.vector.reduce_sum(partials[:, b:b + 1], ee[:, :], axis=AX.X)
                # unnormalized ci: (1,256) psum accumulated over chunks
                pci = pci_pool.tile([1, 256], F32, name=f"pci{b}", tag="pci")
                for i in range(NT):
                    nc.tensor.matmul(pci[:, :], ee[:, i:i + 1], hn_tiles[i][:, :],
                                     start=(i == 0), stop=(i == NT - 1))
                # route the (1,256) ci row into columns of (128,8) psum tiles
                cis = sm_pool.tile([1, 256], BF16, name=f"cis{b}", tag="cis")
                nc.vector.tensor_copy(cis[:, :], pci[:, :])
                nc.tensor.matmul(pciTx[:, b:b + 1], cis[0:1, 0:128],
                                 onescol[0:1, 0:1], start=True, stop=True)
                nc.tensor.matmul(pciTx[:, 8 + b:8 + b + 1], cis[0:1, 128:256],
                                 onescol[0:1, 0:1], start=True, stop=True)

          # ---- phase A wrap-up ----
          with tc.tile_pool(name="wrap", bufs=2) as wr_pool, \
               tc.tile_pool(name="pwr", bufs=1, space="PSUM") as pwr_pool:
              # S_b = sum over partitions of partials[:, b] via ones matmul
              pb16 = wr_pool.tile([128, 8], BF16, name="pb16", tag="pb16")
              nc.vector.tensor_copy(pb16[:, :], partials[:, :])
              psums = pwr_pool.tile([1, 8], F32, name="psums", tag="psums")
              nc.tensor.matmul(psums[:, :], onescol[:, :], pb16[:, :],
                               start=True, stop=True)
              rS = wr_pool.tile([1, 8], F32, name="rS", tag="rS")
              nc.vector.reciprocal(rS[:, :], psums[:, :])
              rSb = wr_pool.tile([1, 8], BF16, name="rSb", tag="rSb")
              nc.vector.tensor_copy(rSb[:, :], rS[:, :])
              prS = pwr_pool.tile([128, 8], F32, name="prS", tag="prS")
              nc.tensor.matmul(prS[:, :], onesrow[:, :], rSb[:, :],
                               start=True, stop=True)
              rSs = wr_pool.tile([128, 8], F32, name="rSs", tag="rSs")
              nc.vector.tensor_copy(rSs[:, :], prS[:, :])
              # normalize ci columns -> ciT (128, 16) bf16
              # (DVE tensor_tensor can read at most one PSUM operand)
              nc.vector.tensor_mul(ciT[:, 0:8], pciT0, rSs[:, :])
              nc.vector.tensor_mul(ciT[:, 8:16], pciT1, rSs[:, :])
              # s0T = tanh(W_init @ h0 + b_init) in T layout
              for j in range(2):
                  ps0 = pwr_pool.tile([128, 8], F32, name=f"ps0{j}", tag="ps0")
                  nc.tensor.matmul(ps0[:, :], winitT[:, j * 128:j * 128 + 128],
                                   h0T[:, 0:8], start=True, stop=False)
                  nc.tensor.matmul(ps0[:, :], winitT[:, 256 + j * 128:256 + j * 128 + 128],
                                   h0T[:, 8:16], start=False, stop=True)
                  nc.scalar.activation(s0T[:, bass.ts(j, 8)], ps0[:, :], AF.Tanh,
                                       bias=binit[:, j:j + 1])
              # constgT = ci @ Wg_ci.T + biases (T layout, 6 f-tiles)
              for jj in range(6):
                  pcg = pwr_pool.tile([128, 8], F32, name=f"pcg{jj}", tag="pcg")
                  nc.tensor.matmul(pcg[:, :], wgciT[:, jj * 128:jj * 128 + 128],
                                   ciT[:, 0:8], start=True, stop=False)
                  nc.tensor.matmul(pcg[:, :], wgciT[:, 768 + jj * 128:768 + jj * 128 + 128],
                                   ciT[:, 8:16], start=False, stop=True)
                  nc.scalar.activation(constgT[:, bass.ts(jj, 8)], pcg[:, :],
                                       AF.Identity, bias=biasgT[:, jj:jj + 1])

        # ================= Phase B =================
        # PSUM accumulation-group discipline: start_tensor_calc=True wipes
        # the has-written state for the WHOLE bank, so a region must never
        # be accumulated into after another group opened in its bank.
        # Every region below is written by one CONSECUTIVE group and only
        # read afterwards.  Layout within the per-step (128,128) f32 bank:
        #   [0:32)   przs  rz gates, s contribution
        #   [32:48)  pnis  gh_n (s contribution)
        #   [48:80)  pyrz  rz gates, y contribution
        #   [80:96)  pyn   i_n y contribution
        #   [96:104) prn   RNN pre-activation
        #   [104:112) pbb  1/S broadcast   row0 [112:120) ps: sum(ey)
        outv = out_all.rearrange("p (b t) -> p b t", t=L)
        with tc.tile_pool(name="pb", bufs=6, space="PSUM") as pb_pool, \
             tc.tile_pool(name="sbb", bufs=8) as sbb_pool, \
             tc.tile_pool(name="stp", bufs=6) as st_pool:

            sT = s0T
            yT = None
            pend = None  # (ey tile, psum tile, t) awaiting softmax tail

            for t in range(L):
                pbt = pb_pool.tile([128, 128], F32, name=f"pbt{t}", tag="pb")
                # --- gate matmuls, s contributions (closed groups) ---
                for jj in range(4):
                    sl = pbt[:, jj * 8:jj * 8 + 8]
                    nc.tensor.matmul(sl, wgsT[:, jj * 128:jj * 128 + 128],
                                     sT[:, 0:8], start=True, stop=False)
                    nc.tensor.matmul(sl, wgsT[:, 768 + jj * 128:768 + jj * 128 + 128],
                                     sT[:, 8:16], start=False, stop=True)
                for jj in range(2):
                    sl = pbt[:, 32 + jj * 8:32 + jj * 8 + 8]
                    nc.tensor.matmul(sl, wgsT[:, (4 + jj) * 128:(4 + jj) * 128 + 128],
                                     sT[:, 0:8], start=True, stop=False)
                    nc.tensor.matmul(sl, wgsT[:, 768 + (4 + jj) * 128:768 + (4 + jj) * 128 + 128],
                                     sT[:, 8:16], start=False, stop=True)

                # rzin = przs + const gates; runs during the softmax tail
                rzin = sbb_pool.tile([128, 32], F32, name=f"rzin{t}", tag="rzin")
                nc.vector.tensor_add(rzin[:, :], pbt[:, 0:32], constgT[:, 0:32])

                # --- previous step's softmax tail ---
                if pend is not None:
                    ey_p, pbt_p, t_p = pend
                    ps = pbt_p[0:1, 112:120]
                    nc.tensor.matmul(ps, onescol[:, :], ey_p[:, :],
                                     start=True, stop=True)
                    rsb = sbb_pool.tile([1, 8], BF16, name=f"rsb{t}", tag="rsb")
                    with nc.allow_low_precision(reason="1/S is rounded to bf16 for the broadcast matmul anyway"):
                        nc.vector.reciprocal(rsb[:, :], ps)
                    pbb = pbt_p[:, 104:112]
                    nc.tensor.matmul(pbb, onesrow[:, :], rsb[:, :],
                                     start=True, stop=True)
                    nc.vector.tensor_mul(outv[:, :, t_p], ey_p[:, :], pbb)
                    yT = outv[:, :, t_p]
                    pend = None

                # --- gate matmuls, y contributions (own closed groups) ---
                if yT is not None:
                    for jj in range(4):
                        nc.tensor.matmul(pbt[:, 48 + jj * 8:48 + jj * 8 + 8],
                                         wgyT[:, jj * 128:jj * 128 + 128],
                                         yT, start=True, stop=True)
                    for jj in range(2):
                        nc.tensor.matmul(pbt[:, 80 + jj * 8:80 + jj * 8 + 8],
                                         wgyT[:, (4 + jj) * 128:(4 + jj) * 128 + 128],
                                         yT, start=True, stop=True)

                # --- GRU elementwise (T layout) ---
                if yT is not None:
                    # r half first: it gates the critical n-path
                    rzin2 = sbb_pool.tile([128, 32], F32, name=f"rzin2{t}", tag="rzin2")
                    nc.vector.tensor_add(rzin2[:, 0:16], rzin[:, 0:16],
                                         pbt[:, 48:64])
                    nc.vector.tensor_add(rzin2[:, 16:32], rzin[:, 16:32],
                                         pbt[:, 64:80])
                else:
                    rzin2 = rzin
                # sigmoid emulated as 0.5*tanh(0.5x)+0.5 (keeps everything in
                # act table set 0, which also holds exp for the softmax).
                # r first: it gates the n-path; z's ops fill engine bubbles.
                trz = sbb_pool.tile([128, 32], F32, name=f"trz{t}", tag="trz")
                nc.scalar.activation(trz[:, 0:16], rzin2[:, 0:16], AF.Tanh,
                                     scale=0.5)
                nc.scalar.activation(trz[:, 16:32], rzin2[:, 16:32], AF.Tanh,
                                     scale=0.5)
                # n = tanh(r*ghn + i_n) with r = 0.5*trz+0.5 expands to
                # tanh(0.5*(trz*ghn + ghn + 2*i_n)); the 0.5 rides the ACT
                # scale and the 2x is pre-folded into wgciT/wgyT/biasgT
                # n-parts host-side, so q = ghn + 2*i_n builds OFF the chain
                # while the ACT computes trz, leaving only two chain ops.
                sig = sbb_pool.tile([128, 32], F32, name=f"sig{t}", tag="sig")
                q1 = sbb_pool.tile([128, 16], F32, name=f"q1_{t}", tag="q1")
                nc.vector.tensor_add(q1[:, :], pbt[:, 32:48], constgT[:, 32:48])
                if yT is None:
                    q = q1
                else:
                    q = sbb_pool.tile([128, 16], F32, name=f"q{t}", tag="q")
                    nc.vector.tensor_add(q[:, :], q1[:, :], pbt[:, 80:96])
                ns = sbb_pool.tile([128, 16], F32, name=f"ns{t}", tag="ns")
                rh = sbb_pool.tile([128, 16], F32, name=f"rh{t}", tag="rh")
                nc.vector.tensor_mul(rh[:, :], trz[:, 0:16], pbt[:, 32:48])
                nc.vector.tensor_add(ns[:, :], rh[:, :], q[:, :])
                # sTn = (1-z)*nn + z*sT: both z terms precompute on the
                # DVE during the nn activation (z and the old state are
                # ready), leaving only two chain ops after nn
                nc.vector.tensor_scalar(sig[:, 16:32], trz[:, 16:32], 0.5, 0.5,
                                        OP.mult, OP.add)
                izq = sbb_pool.tile([128, 16], F32, name=f"izq{t}", tag="izq")
                nc.vector.tensor_scalar(izq[:, :], trz[:, 16:32], -0.5, 0.5,
                                        OP.mult, OP.add)
                zs = sbb_pool.tile([128, 16], F32, name=f"zs{t}", tag="zs")
                nc.vector.tensor_mul(zs[:, :], sig[:, 16:32], sT[:, :])
                nn = sbb_pool.tile([128, 16], BF16, name=f"nn{t}", tag="nn")
                nc.scalar.activation(nn[:, :], ns[:, :], AF.Tanh, scale=0.5)
                zn = sbb_pool.tile([128, 16], F32, name=f"zn{t}", tag="zn")
                nc.vector.tensor_mul(zn[:, :], izq[:, :], nn[:, :])
                sTn = st_pool.tile([128, 16], BF16, name=f"sT{t}", tag="sT")
                nc.vector.tensor_add(sTn[:, :], zn[:, :], zs[:, :])

                # --- RNN cell + exp (one consecutive group) ---
                # ci/y parts first: they have no sTn dependence, so they
                # execute during the GRU elementwise; only the two s-part
                # matmuls remain after sTn lands.  No foreign start=True
                # touches this bank while the group is open.
                prn = pbt[:, 96:104]
                nc.tensor.matmul(prn, wrciT[:, 0:128], ciT[:, 0:8],
                                 start=True, stop=False)
                nc.tensor.matmul(prn, wrciT[:, 128:256], ciT[:, 8:16],
                                 start=False, stop=False)
                if yT is not None:
                    nc.tensor.matmul(prn, wryT[:, :], yT,
                                     start=False, stop=False)
                nc.tensor.matmul(prn, wrsT[:, 0:128], sTn[:, 0:8],
                                 start=False, stop=False)
                nc.tensor.matmul(prn, wrsT[:, 128:256], sTn[:, 8:16],
                                 start=False, stop=True)
                vv = sbb_pool.tile([128, 8], F32, name=f"vv{t}", tag="vv")
                nc.scalar.activation(vv[:, :], prn, AF.Tanh,
                                     bias=biasrT[:, :])
                # native exp: vv = tanh(..) is in [-1,1], where the exp table
                # is accurate enough (the wide-domain phase A softmax keeps
                # the tanh-identity form)
                ey = sbb_pool.tile([128, 8], BF16, name=f"ey{t}", tag="ey")
                nc.scalar.activation(ey[:, :], vv[:, :], AF.Exp)

                pend = (ey, pbt, t)
                sT = sTn

            # final step's softmax tail
            ey_p, pbt_p, t_p = pend
            ps = pbt_p[0:1, 112:120]
            nc.tensor.matmul(ps, onescol[:, :], ey_p[:, :], start=True, stop=True)
            rsb = sbb_pool.tile([1, 8], BF16, name="rsbF", tag="rsb")
            with nc.allow_low_precision(reason="1/S is rounded to bf16 for the broadcast matmul anyway"):
                nc.vector.reciprocal(rsb[:, :], ps)
            pbb = pbt_p[:, 104:112]
            nc.tensor.matmul(pbb, onesrow[:, :], rsb[:, :], start=True, stop=True)
            nc.vector.tensor_mul(outv[:, :, t_p], ey_p[:, :], pbb)

        nc.sync.dma_start(out_d[:, :], out_all[:, :])

    nc.compile()
    return nc


def _pack_weights(inputs):
    f = np.float32
    bf = ml_dtypes.bfloat16
    W_h_a = np.asarray(inputs["W_h_a"], f)
    W_a = np.asarray(inputs["W_a"], f)
    W_init = np.asarray(inputs["W_init"], f)
    b_init = np.asarray(inputs["b_init"], f)
    W_ih_g = np.asarray(inputs["W_ih_g"], f)
    W_hh_g = np.asarray(inputs["W_hh_g"], f)
    b_ih_g = np.asarray(inputs["b_ih_g"], f)
    b_hh_g = np.asarray(inputs["b_hh_g"], f)
    W_ih_r = np.asarray(inputs["W_ih_r"], f)
    W_hh_r = np.asarray(inputs["W_hh_r"], f)
    b_ih_r = np.asarray(inputs["b_ih_r"], f)
    b_hh_r = np.asarray(inputs["b_hh_r"], f)

    assert not np.any(b_hh_g[512:]), "nonzero b_hh_g n-part not supported"

    def split2(m):  # (256, X) -> (128, 2X), k-chunks side by side
        return np.concatenate([m[0:128], m[128:256]], axis=1)

    wk = {}
    wk["whaT"] = split2(W_h_a.T).astype(bf)
    wk["wa1r"] = np.tile(W_a[0, :256][None, :], (128, 1)).astype(bf)
    wk["winitT"] = split2(W_init.T).astype(bf)
    wk["binit"] = np.ascontiguousarray(b_init.reshape(2, 128).T)
    wk["wgsT"] = split2(W_hh_g.T).astype(bf)
    # n-parts doubled: the GRU n-path computes tanh(0.5*(trz*ghn+ghn+2*i_n))
    wgy = W_ih_g[:, 256:384].T.copy()
    wgy[:, 512:768] *= 2.0
    wk["wgyT"] = wgy.astype(bf)
    wgci = W_ih_g[:, 0:256].T.copy()
    wgci[:, 512:768] *= 2.0
    wk["wgciT"] = split2(wgci).astype(bf)
    wk["wrsT"] = split2(W_ih_r[:, 256:512].T).astype(bf)
    wk["wryT"] = W_hh_r.T.astype(bf)
    wk["wrciT"] = split2(W_ih_r[:, 0:256].T).astype(bf)
    bias_g = b_ih_g + np.concatenate([b_hh_g[:512], np.zeros(256, f)])
    bias_g = bias_g.copy()
    bias_g[512:768] *= 2.0
    wk["biasgT"] = np.ascontiguousarray(bias_g.reshape(6, 128).T)
    wk["biasrT"] = np.ascontiguousarray((b_ih_r + b_hh_r).reshape(128, 1))
    wk["ident"] = np.eye(128, dtype=bf)
    wk["onescol"] = np.ones((128, 1), bf)
    wk["onesrow"] = np.ones((1, 128), bf)
    return {k: np.ascontiguousarray(v) for k, v in wk.items()}


def run(inputs, trace=False):
    from concourse import bass_utils

    assert int(inputs["out_len"]) == L
    if "nc" not in _CACHE:
        _CACHE["nc"] = _build_program()
    nc = _CACHE["nc"]

    wk = _pack_weights(inputs)
    h = np.asarray(inputs["h"], np.float32).astype(ml_dtypes.bfloat16)
    in_maps = []
    for c in range(NC):
        m = dict(wk)
        m["h"] = np.ascontiguousarray(h[c * BL:(c + 1) * BL])
        in_maps.append(m)

    res = bass_utils.run_bass_kernel_spmd(
        nc, in_maps, core_ids=list(range(NC)), trace=trace)

    out = np.empty((B, L, DO), np.float32)
    for c in range(NC):
        r = np.asarray(res.results[c]["out"]).astype(np.float32)
        r = r.reshape(128, BL, L)
        out[c * BL:(c + 1) * BL] = r.transpose(1, 2, 0)
    return out, res


def kernel(**inputs):
    out, _ = run(inputs, trace=False)
    return out

